# revision 1
# baseline (speedup 1.0000x reference)
"""MoE (Gemma-style 8-expert top-2) Trainium2 kernel.

Strategy (expert-parallel over 8 NeuronCores):
  - Host: merge duplicate (token, expert) assignments, build per-expert token
    lists, gather+transpose x into xT_e [H, C] per expert (zero-padded to a
    common capacity C).  This is the "dispatch" all-to-all done host-side,
    which the full-input/full-output contract allows.  Weights are converted
    to fp16 and prepacked per 128-wide output tile so every device DMA is a
    simple 2D contiguous descriptor.
  - Device (per core e): dense expert MLP on its C tokens, all in transposed
    layout so every matmul uses natural weight layouts with zero on-device
    transposes:
        gateT[i, c] = sum_h Wg[h,i] * xT[h,c]     (weights stationary)
        upT   likewise
        hT    = gelu_tanh(gateT) * upT            [I, C]  (fp16 in SBUF)
        yT[h, c] = sum_i Wd[i,h] * hT[i,c]        [H, C]
    Matmul operands are fp16 (full PE rate, FWL weight loads); accumulation
    is fp32 in PSUM.  A short burst of dummy matmuls at kernel start warms
    the PE HAM clock-gate while the first DMAs land.
  - Host: combine — out[t] += route[t,e] * yT_e[:, pos].T  (the "combine"
    all-to-all), with route exactly matching the reference's scatter-add.
"""

import numpy as np

import concourse.bass as bass
import concourse.mybir as mybir
import concourse.tile as tile
from concourse import bacc


def _install_ntff_hook_shim():
    """The agent image's `antenv` lacks `axon_hooks`, which bass_utils
    imports unconditionally when tracing under axon.  Provide the module
    and register the ctypes-based NTFF profile hook so BASS_TRACE=1 yields
    real HW profiles.  Degrades silently if anything is missing."""
    import sys
    import types

    try:
        import antenv

        try:
            from antenv import axon_hooks  # noqa: F401

            return
        except ImportError:
            pass
        mod = types.ModuleType("antenv.axon_hooks")
        mod._hook = None
        mod.set_axon_ntff_profile_hook = lambda h: setattr(mod, "_hook", h)
        mod.get_axon_ntff_profile_hook = lambda: mod._hook
        sys.modules["antenv.axon_hooks"] = mod
        antenv.axon_hooks = mod
        import os

        so_path = "/opt/axon/libaxon_pjrt.so"
        if os.path.exists(so_path):
            from trn_agent_boot.trn_boot import _ntff_profile_via_ctypes

            mod._hook = _ntff_profile_via_ctypes(so_path)
    except Exception:
        pass


_install_ntff_hook_shim()

from concourse.bass_utils import run_bass_kernel_spmd

H = 2048
I = 4096
E = 8
P = 128
KH = H // P  # 16 contraction chunks for gate/up
MI = I // P  # 32 output tiles of I
KI = I // P  # 32 contraction chunks for down
MH = H // P  # 16 output tiles of H
F32 = mybir.dt.float32
F16 = mybir.dt.float16

# Results of the last device run (for test harnesses to inspect profiling).
LAST_RESULTS = None

_PROGRAM_CACHE: dict[int, "bass.Bass"] = {}


def _build_program(C: int) -> "bass.Bass":
    """Bass program for one core: expert MLP on C tokens (transposed layout)."""
    assert C % 8 == 0 and 256 <= C <= 512

    nc = bacc.Bacc("TRN2", target_bir_lowering=False)

    # Host-prepacked inputs: each [t, :, :] slab is one SBUF tile, contiguous.
    xT = nc.dram_tensor("xT", [H, C], F16, kind="ExternalInput")
    Wg = nc.dram_tensor("Wg", [MI, P, KH * P], F16, kind="ExternalInput")
    Wu = nc.dram_tensor("Wu", [MI, P, KH * P], F16, kind="ExternalInput")
    Wd = nc.dram_tensor("Wd", [MH, P, KI * P], F16, kind="ExternalInput")
    yT = nc.dram_tensor("yT", [H, C], F32, kind="ExternalOutput")

    xT_r = xT.rearrange("(k p) c -> p k c", p=P)  # [128, 16, C]
    yT_r = yT.rearrange("(m p) c -> p m c", p=P)  # [128, 16, C]
    Wg_a, Wu_a, Wd_a = Wg.ap(), Wu.ap(), Wd.ap()

    gelu = mybir.ActivationFunctionType.Gelu_apprx_tanh

    with tile.TileContext(nc) as tc:
        with (
            tc.tile_pool(name="xpool", bufs=1) as xpool,
            tc.tile_pool(name="hpool", bufs=1) as hpool,
            tc.tile_pool(name="wpool", bufs=6) as wpool,
            tc.tile_pool(name="tpool", bufs=3) as tpool,
            tc.tile_pool(name="warm", bufs=1) as warm_pool,
            tc.tile_pool(name="psum", bufs=2) as _psum_unused,  # keep name stable
            tc.tile_pool(name="psum2", bufs=2, space="PSUM") as psum_pool,
            tc.tile_pool(name="psumw", bufs=1, space="PSUM") as psum_warm,
        ):
            # --- PE warm-up: dummy matmuls over zeros while first DMAs land
            wz = warm_pool.tile([P, P], F16)
            xz = warm_pool.tile([P, C], F16)
            nc.vector.memset(wz, 0.0)
            nc.vector.memset(xz, 0.0)
            psum_w = psum_warm.tile([P, C], F32, tag="warm")
            for _ in range(16):
                nc.tensor.matmul(psum_w, wz, xz, start=True, stop=True)

            # x resident in SBUF: [128, 16, C] fp16
            xsb = xpool.tile([P, KH, C], F16)
            # first chunks early (and fine-grained) so m=0 matmuls start ASAP
            nc.sync.dma_start(out=xsb[:, 0:2, :], in_=xT_r[:, 0:2, :])
            nc.sync.dma_start(out=xsb[:, 2:4, :], in_=xT_r[:, 2:4, :])

            # h resident in SBUF: [128, 32, C] fp16
            hsb = hpool.tile([P, KI, C], F16)

            def load_w(dram_ap, t, tag, splits=1):
                wt = wpool.tile([P, KH * P], F16, tag=tag, name=f"w_{tag}_{t}")
                step = (KH * P) // splits
                for s in range(splits):
                    nc.sync.dma_start(
                        out=wt[:, s * step : (s + 1) * step],
                        in_=dram_ap[t, :, s * step : (s + 1) * step],
                    )
                return wt.rearrange("p (k i) -> p k i", i=P)

            # ---- Phase 1: gateT/upT -> hT, one I-tile (128 rows) at a time
            for m in range(MI):
                wg_t = load_w(Wg_a, m, "wg")
                wu_t = load_w(Wu_a, m, "wu")
                if m == 0:
                    # rest of x arrives while m=0 computes
                    for q in range(1, 4):
                        nc.sync.dma_start(
                            out=xsb[:, 4 * q : 4 * (q + 1), :],
                            in_=xT_r[:, 4 * q : 4 * (q + 1), :],
                        )

                psum_g = psum_pool.tile([P, C], F32, tag="g")
                psum_u = psum_pool.tile([P, C], F32, tag="u")
                for k in range(KH):
                    nc.tensor.matmul(
                        psum_g,
                        wg_t[:, k, :],
                        xsb[:, k, :],
                        start=(k == 0),
                        stop=(k == KH - 1),
                    )
                for k in range(KH):
                    nc.tensor.matmul(
                        psum_u,
                        wu_t[:, k, :],
                        xsb[:, k, :],
                        start=(k == 0),
                        stop=(k == KH - 1),
                    )
                tg = tpool.tile([P, C], F32, tag="gelu")
                nc.scalar.activation(tg, psum_g, gelu)
                nc.vector.tensor_mul(hsb[:, m, :], tg, psum_u)

            # ---- Phase 2: downT -> yT, one H-tile (128 rows) at a time
            for m2 in range(MH):
                wd_t = wpool.tile([P, KI * P], F16, tag="wd", name=f"w_wd_{m2}")
                nc.sync.dma_start(out=wd_t, in_=Wd_a[m2])
                wd_v = wd_t.rearrange("p (k i) -> p k i", i=P)
                if m2 < MH - 1:
                    psum_d = psum_pool.tile([P, C], F32, tag="d")
                    for k2 in range(KI):
                        nc.tensor.matmul(
                            psum_d,
                            wd_v[:, k2, :],
                            hsb[:, k2, :],
                            start=(k2 == 0),
                            stop=(k2 == KI - 1),
                        )
                    ysb = tpool.tile([P, C], F32, tag="y")
                    nc.vector.tensor_copy(ysb, psum_d)
                    nc.sync.dma_start(out=yT_r[:, m2, :], in_=ysb)
                else:
                    # last tile: two half-width accumulations so the first
                    # half's copy+DMA hides under the second half's matmuls
                    half = C // 2
                    psum_d = psum_pool.tile([P, half], F32, tag="d")
                    psum_e = psum_pool.tile([P, C - half], F32, tag="g")
                    for k2 in range(KI):
                        nc.tensor.matmul(
                            psum_d,
                            wd_v[:, k2, :],
                            hsb[:, k2, 0:half],
                            start=(k2 == 0),
                            stop=(k2 == KI - 1),
                        )
                    ysb_a = tpool.tile([P, half], F32, tag="y")
                    nc.vector.tensor_copy(ysb_a, psum_d)
                    nc.sync.dma_start(out=yT_r[:, m2, 0:half], in_=ysb_a)
                    for k2 in range(KI):
                        nc.tensor.matmul(
                            psum_e,
                            wd_v[:, k2, :],
                            hsb[:, k2, half:C],
                            start=(k2 == 0),
                            stop=(k2 == KI - 1),
                        )
                    ysb_b = tpool.tile([P, C - half], F32, tag="y")
                    nc.vector.tensor_copy(ysb_b, psum_e)
                    nc.sync.dma_start(out=yT_r[:, m2, half:C], in_=ysb_b)

    nc.compile()
    return nc


def _get_program(C: int) -> "bass.Bass":
    if C not in _PROGRAM_CACHE:
        _PROGRAM_CACHE[C] = _build_program(C)
    return _PROGRAM_CACHE[C]


def _prep_w_gu(w):  # [H, I] f32 -> [MI, P, KH*P] 16-bit, per-tile contiguous
    return np.ascontiguousarray(
        w.astype(np.float16).reshape(KH, P, MI, P).transpose(2, 1, 0, 3)
    ).reshape(MI, P, KH * P)


def _prep_w_d(w):  # [I, H] f32 -> [MH, P, KI*P] 16-bit
    return np.ascontiguousarray(
        w.astype(np.float16).reshape(KI, P, MH, P).transpose(2, 1, 0, 3)
    ).reshape(MH, P, KI * P)


def kernel(x, selected_experts, routing_weights, Wg, Wu, Wd):
    global LAST_RESULTS
    x = np.asarray(x, dtype=np.float32)
    se = np.asarray(selected_experts).astype(np.int64)
    rw = np.asarray(routing_weights).astype(np.float32)
    Wg = np.asarray(Wg, dtype=np.float32)
    Wu = np.asarray(Wu, dtype=np.float32)
    Wd = np.asarray(Wd, dtype=np.float32)

    T, K = se.shape
    assert x.shape == (T, H) and Wg.shape == (E, H, I) and Wd.shape == (E, I, H)

    # Dense route matrix, identical to the reference's scatter-add (merges
    # duplicate expert picks within a token by summing their weights).
    flat_t = np.repeat(np.arange(T), K)
    flat_e = se.ravel()
    route = np.zeros((T, E), np.float32)
    np.add.at(route, (flat_t, flat_e), rw.ravel())
    present = np.zeros((T, E), bool)
    present[flat_t, flat_e] = True

    idx_lists = [np.nonzero(present[:, e])[0] for e in range(E)]
    chunked = [
        [ix[s : s + 512] for s in range(0, max(len(ix), 1), 512)] for ix in idx_lists
    ]
    n_pass = max(len(ch) for ch in chunked)

    out = np.zeros((T, H), np.float32)
    for p in range(n_pass):
        parts = [ch[p] if p < len(ch) else np.empty(0, np.int64) for ch in chunked]
        max_count = max(len(ix) for ix in parts)
        C = max(256, min(512, -(-max(max_count, 1) // 8) * 8))
        nc = _get_program(C)
        in_maps = []
        for e in range(E):
            ix = parts[e]
            xT_e = np.zeros((H, C), np.float16)
            if len(ix):
                xT_e[:, : len(ix)] = x[ix].T.astype(np.float16)
            in_maps.append(
                {
                    "xT": xT_e,
                    "Wg": _prep_w_gu(Wg[e]),
                    "Wu": _prep_w_gu(Wu[e]),
                    "Wd": _prep_w_d(Wd[e]),
                }
            )
        res = run_bass_kernel_spmd(nc, in_maps, core_ids=list(range(E)))
        LAST_RESULTS = res
        for e in range(E):
            ix = parts[e]
            if len(ix) == 0:
                continue
            yT_e = res.results[e]["yT"]  # [H, C]
            out[ix] += route[ix, e][:, None] * yT_e[:, : len(ix)].T
    return out



# revision 7
# speedup vs baseline: 1.0115x; 1.0115x over previous
"""MoE (Gemma-style 8-expert top-2) Trainium2 kernel — intermediate-sharded.

Strategy (tensor-parallel over the intermediate dim, 8 NeuronCores):
  - Host: merge duplicate (token, expert) assignments, build per-expert token
    lists, gather+transpose x into a packed xT stream (one contiguous block
    per expert, zero-padded to a multiple of 4).  Weights are fp16 and
    prepacked per core: core j owns columns [j*512, (j+1)*512) of the
    intermediate dim for ALL 8 experts, so every core executes the exact
    same (perfectly balanced) sequence of matmuls — sum_e 192*C_e cycles —
    instead of being held hostage by the most-loaded expert.
  - Device (core j), for each expert e with C_e tokens, all transposed
    layout so matmuls use natural weight layouts:
        gateT[i, c] = sum_h Wg[h, j*512+i] * xT[h, c]   (i in [0,512))
        upT   likewise
        hT    = gelu_tanh(gateT) * upT        [512, C]  fp16 in SBUF
        yT[h, c] = sum_i Wd[j*512+i, h] * hT[i, c]      [2048, C] partial!
    Emission is software-pipelined: the down-projection of expert e-1 is
    woven between the gate/up tiles of expert e so the PE never waits for
    the gelu/mul (DVE/ACT) results of the expert it just finished.
  - Host: combine — sum the 8 per-core partial yT (fp32), then
    out[t] += route[t,e] * yT_e[:, pos].T with route matching the
    reference's scatter-add exactly.
"""

import numpy as np

import concourse.bass as bass
import concourse.mybir as mybir
import concourse.tile as tile
from concourse import bacc


def _install_ntff_hook_shim():
    """The agent image's `antenv` lacks `axon_hooks`, which bass_utils
    imports unconditionally when tracing under axon.  Provide the module
    and register the ctypes-based NTFF profile hook so BASS_TRACE=1 yields
    real HW profiles.  Degrades silently if anything is missing."""
    import sys
    import types

    try:
        import antenv

        try:
            from antenv import axon_hooks  # noqa: F401

            return
        except ImportError:
            pass
        mod = types.ModuleType("antenv.axon_hooks")
        mod._hook = None
        mod.set_axon_ntff_profile_hook = lambda h: setattr(mod, "_hook", h)
        mod.get_axon_ntff_profile_hook = lambda: mod._hook
        sys.modules["antenv.axon_hooks"] = mod
        antenv.axon_hooks = mod
        import os

        so_path = "/opt/axon/libaxon_pjrt.so"
        if os.path.exists(so_path):
            from trn_agent_boot.trn_boot import _ntff_profile_via_ctypes

            mod._hook = _ntff_profile_via_ctypes(so_path)
    except Exception:
        pass


_install_ntff_hook_shim()

from concourse.bass_utils import run_bass_kernel_spmd

H = 2048
I = 4096
E = 8
P = 128
NCORES = 8
ISL = I // NCORES  # 512-wide intermediate slice per core
MI = ISL // P  # 4 gate/up output tiles per expert
KH = H // P  # 16 contraction chunks for gate/up
KI = ISL // P  # 4 contraction chunks for down
MH = H // P  # 16 down output tiles
CMAX = 504  # max token-columns per work item (PSUM bank = 512 fp32)
CPAD = 4
WBLK = MI * KH * P  # 8192 weight cols per expert (gate/up); same for down
F32 = mybir.dt.float32
F16 = mybir.dt.float16

# Results of the last device run (for test harnesses to inspect profiling).
LAST_RESULTS = None

_PROGRAM_CACHE: dict[tuple, "bass.Bass"] = {}


def _build_program(items: tuple[tuple[int, int], ...]) -> "bass.Bass":
    """Bass program for one core: for each (expert, C) item, the full expert
    MLP on its I-slice.  Identical across cores (weights differ)."""
    n = len(items)
    xcols = sum(KH * c for _, c in items)
    ycols = sum(MH * c for _, c in items)

    nc = bacc.Bacc("TRN2", target_bir_lowering=False)

    XT = nc.dram_tensor("XT", [P, xcols], F16, kind="ExternalInput")
    WG = nc.dram_tensor("WG", [P, E * WBLK], F16, kind="ExternalInput")
    WU = nc.dram_tensor("WU", [P, E * WBLK], F16, kind="ExternalInput")
    WD = nc.dram_tensor("WD", [P, E * WBLK], F16, kind="ExternalInput")
    Y = nc.dram_tensor("Y", [P, ycols], F16, kind="ExternalOutput")

    XT_a, WG_a, WU_a, WD_a, Y_a = XT.ap(), WG.ap(), WU.ap(), WD.ap(), Y.ap()

    gelu = mybir.ActivationFunctionType.Gelu_apprx_tanh

    xoffs, yoffs = [], []
    xo = yo = 0
    for _, c in items:
        xoffs.append(xo)
        yoffs.append(yo)
        xo += KH * c
        yo += MH * c

    with tile.TileContext(nc) as tc:
        with (
            tc.tile_pool(name="xpool", bufs=2) as xpool,
            tc.tile_pool(name="hpool", bufs=2) as hpool,
            tc.tile_pool(name="wgpool", bufs=2) as wgpool,
            tc.tile_pool(name="wupool", bufs=2) as wupool,
            tc.tile_pool(name="wdpool", bufs=2) as wdpool,
            tc.tile_pool(name="ypool", bufs=2) as ypool,
            tc.tile_pool(name="gpool", bufs=3) as gpool,
            tc.tile_pool(name="warm", bufs=1) as warm_pool,
            tc.tile_pool(name="psumgu", bufs=2, space="PSUM") as psum_gu,
            tc.tile_pool(name="psumd", bufs=2, space="PSUM") as psum_d_pool,
            tc.tile_pool(name="psumw", bufs=1, space="PSUM") as psum_warm,
        ):
            # --- PE warm-up: dummy matmuls on zeros trip the HAM clock-gate
            # to 8/8 while the first DMAs land (~3.4us budget).
            wz = warm_pool.tile([P, P], F16)
            xz = warm_pool.tile([P, P], F16)
            nc.vector.memset(wz, 0.0)
            nc.vector.memset(xz, 0.0)
            psum_w = psum_warm.tile([P, P], F32, tag="warm")
            for _ in range(48):
                nc.tensor.matmul(psum_w, wz, xz, start=True, stop=True)

            tiles = [None] * n  # per item: (xsb, wg_v, wu_v, wd_v)
            ysbs = [None] * n

            def emit_loads(i, first):
                e, C = items[i]
                xsb = xpool.tile([P, KH, C], F16, tag="x", name=f"x_{i}")
                xflat = xsb.rearrange("p k c -> p (k c)")
                xa = XT_a[:, xoffs[i] : xoffs[i] + KH * C]
                # chunked so the first matmuls of the item start before the
                # whole tile has landed (Tile tracks sub-range deps)
                bounds = [0, 1, 2, 4, 8, 16] if first else [0, 4, 10, 16]
                for a, b in zip(bounds, bounds[1:]):
                    nc.sync.dma_start(
                        out=xflat[:, a * C : b * C], in_=xa[:, a * C : b * C]
                    )

                def wtile(pool, src, tag, splits):
                    wt = pool.tile([P, WBLK], F16, tag=tag, name=f"{tag}_{i}")
                    sa = src[:, e * WBLK : (e + 1) * WBLK]
                    step = WBLK // splits
                    for s in range(splits):
                        nc.sync.dma_start(
                            out=wt[:, s * step : (s + 1) * step],
                            in_=sa[:, s * step : (s + 1) * step],
                        )
                    return wt

                wg_t = wtile(wgpool, WG_a, "wg", 4 if first else 2)
                wu_t = wtile(wupool, WU_a, "wu", 4 if first else 2)
                wd_t = wtile(wdpool, WD_a, "wd", 2 if first else 1)
                tiles[i] = (
                    xsb,
                    wg_t.rearrange("p (m k i) -> p m k i", m=MI, k=KH),
                    wu_t.rearrange("p (m k i) -> p m k i", m=MI, k=KH),
                    wd_t.rearrange("p (m k i) -> p m k i", m=MH, k=KI),
                )

            def emit_down_chunk(i, m2_range):
                """Down-projection tiles m2_range of item i + drains."""
                e, C = items[i]
                wd_v = tiles[i][3]
                hsb = hsbs[i]
                if ysbs[i] is None:
                    ysbs[i] = ypool.tile([P, MH, C], F16, tag="y", name=f"y_{i}")
                ysb = ysbs[i]
                for m2 in m2_range:
                    psum_d = psum_d_pool.tile([P, C], F32, tag="d")
                    for k2 in range(KI):
                        nc.tensor.matmul(
                            psum_d,
                            wd_v[:, m2, k2, :],
                            hsb[:, k2, :],
                            start=(k2 == 0),
                            stop=(k2 == KI - 1),
                        )
                    nc.vector.tensor_copy(ysb[:, m2, :], psum_d)

            def emit_y_dma(i, lo, hi):
                e, C = items[i]
                yflat = ysbs[i].rearrange("p m c -> p (m c)")
                nc.sync.dma_start(
                    out=Y_a[:, yoffs[i] + lo * C : yoffs[i] + hi * C],
                    in_=yflat[:, lo * C : hi * C],
                )

            hsbs = [None] * n
            down_sched = [range(0, 4), range(4, 8), range(8, 12), range(12, 16)]

            emit_loads(0, True)

            for i in range(n):
                e, C = items[i]
                if i + 1 < n:
                    emit_loads(i + 1, False)
                xsb, wg_v, wu_v, _ = tiles[i]
                hsb = hpool.tile([P, KI, C], F16, tag="h", name=f"h_{i}")
                hsbs[i] = hsb
                for m in range(MI):
                    psum_g = psum_gu.tile([P, C], F32, tag="g")
                    psum_u = psum_gu.tile([P, C], F32, tag="u")
                    for k in range(KH):
                        nc.tensor.matmul(
                            psum_g,
                            wg_v[:, m, k, :],
                            xsb[:, k, :],
                            start=(k == 0),
                            stop=(k == KH - 1),
                        )
                    for k in range(KH):
                        nc.tensor.matmul(
                            psum_u,
                            wu_v[:, m, k, :],
                            xsb[:, k, :],
                            start=(k == 0),
                            stop=(k == KH - 1),
                        )
                    tg = gpool.tile([P, C], F32, tag="gelu", name=f"g_{i}_{m}")
                    nc.scalar.activation(tg, psum_g, gelu)
                    nc.vector.tensor_mul(hsb[:, m, :], tg, psum_u)
                    if i > 0:
                        emit_down_chunk(i - 1, down_sched[m])
                        if m == 1:
                            emit_y_dma(i - 1, 0, 8)
                        elif m == 3:
                            emit_y_dma(i - 1, 8, 16)
                # release previous item's tiles happens via pool cycling

            # tail: down-projection of the last item
            for q in range(4):
                emit_down_chunk(n - 1, down_sched[q])
                emit_y_dma(n - 1, 4 * q, 4 * q + 4)

    nc.compile()
    return nc


def _get_program(items) -> "bass.Bass":
    key = tuple(items)
    if key not in _PROGRAM_CACHE:
        _PROGRAM_CACHE[key] = _build_program(key)
    return _PROGRAM_CACHE[key]


def _pack_wgu(w16):
    """[E, H, I] fp16 -> [NCORES, P, E*WBLK]: per core j, expert-block layout
    cols = ((e*MI + m)*KH + k)*P + i  with value Wg[e][k*P+p, j*ISL+m*P+i]."""
    a = w16.reshape(E, KH, P, NCORES, MI, P).transpose(3, 2, 0, 4, 1, 5)
    return np.ascontiguousarray(a).reshape(NCORES, P, E * WBLK)


def _pack_wd(w16):
    """[E, I, H] fp16 -> [NCORES, P, E*WBLK]: cols ((e*MH+m2)*KI+k2)*P + hh
    with value Wd[e][j*ISL + k2*P + p, m2*P + hh]."""
    a = w16.reshape(E, NCORES, KI, P, MH, P).transpose(1, 3, 0, 4, 2, 5)
    return np.ascontiguousarray(a).reshape(NCORES, P, E * WBLK)


def kernel(x, selected_experts, routing_weights, Wg, Wu, Wd):
    global LAST_RESULTS
    x = np.asarray(x, dtype=np.float32)
    se = np.asarray(selected_experts).astype(np.int64)
    rw = np.asarray(routing_weights).astype(np.float32)
    Wg = np.asarray(Wg, dtype=np.float32)
    Wu = np.asarray(Wu, dtype=np.float32)
    Wd = np.asarray(Wd, dtype=np.float32)

    T, K = se.shape
    assert x.shape == (T, H) and Wg.shape == (E, H, I) and Wd.shape == (E, I, H)

    # Dense route matrix, identical to the reference's scatter-add (merges
    # duplicate expert picks within a token by summing their weights).
    flat_t = np.repeat(np.arange(T), K)
    flat_e = se.ravel()
    route = np.zeros((T, E), np.float32)
    np.add.at(route, (flat_t, flat_e), rw.ravel())
    present = np.zeros((T, E), bool)
    present[flat_t, flat_e] = True

    # Work items: (expert, token-index chunk), chunks capped at CMAX columns,
    # padded to a multiple of CPAD.  Largest first (shortest tail last).
    work = []
    for e in range(E):
        ix = np.nonzero(present[:, e])[0]
        for s in range(0, len(ix), CMAX):
            chunk = ix[s : s + CMAX]
            cpad = max(CPAD, -(-len(chunk) // CPAD) * CPAD)
            work.append((e, chunk, cpad))
    work.sort(key=lambda w: -w[2])

    items = tuple((e, c) for e, _, c in work)
    nc = _get_program(items)

    xoffs, yoffs = [], []
    xo = yo = 0
    for _, c in items:
        xoffs.append(xo)
        yoffs.append(yo)
        xo += KH * c
        yo += MH * c

    # --- pack inputs
    XT = np.zeros((P, xo), np.float16)
    for (e, ix, c), xof in zip(work, xoffs):
        blk = np.zeros((P, KH, c), np.float16)
        if len(ix):
            # x[ix].T: [H, Ca] -> [KH, P, Ca] -> [P, KH, Ca]
            blk[:, :, : len(ix)] = (
                x[ix].T.astype(np.float16).reshape(KH, P, len(ix)).transpose(1, 0, 2)
            )
        XT[:, xof : xof + KH * c] = blk.reshape(P, KH * c)

    WGp = _pack_wgu(Wg.astype(np.float16))
    WUp = _pack_wgu(Wu.astype(np.float16))
    WDp = _pack_wd(Wd.astype(np.float16))

    in_maps = [
        {"XT": XT, "WG": WGp[j], "WU": WUp[j], "WD": WDp[j]} for j in range(NCORES)
    ]
    res = run_bass_kernel_spmd(nc, in_maps, core_ids=list(range(NCORES)))
    LAST_RESULTS = res

    # --- combine: sum partial yT across cores, then weighted scatter-add
    Ysum = np.zeros((P, yo), np.float32)
    for j in range(NCORES):
        Ysum += res.results[j]["Y"].astype(np.float32)

    out = np.zeros((T, H), np.float32)
    for (e, ix, c), yof in zip(work, yoffs):
        if len(ix) == 0:
            continue
        blk = Ysum[:, yof : yof + MH * c].reshape(P, MH, c)[:, :, : len(ix)]
        # [p, m2, c] -> [c, m2, p] -> [c, H]
        y_e = np.ascontiguousarray(blk.transpose(2, 1, 0)).reshape(len(ix), H)
        out[ix] += route[ix, e][:, None] * y_e
    return out


# revision 10
# speedup vs baseline: 1.0231x; 1.0115x over previous
"""MoE (Gemma-style 8-expert top-2) Trainium2 kernel — intermediate-sharded.

Strategy (tensor-parallel over the intermediate dim, 8 NeuronCores):
  - Host: merge duplicate (token, expert) assignments, build per-expert token
    lists, gather+transpose x into a packed xT stream (one contiguous block
    per expert, zero-padded to a multiple of 4).  Weights are fp16 and
    prepacked per core: core j owns columns [j*512, (j+1)*512) of the
    intermediate dim for ALL 8 experts, so every core executes the exact
    same (perfectly balanced) sequence of matmuls — sum_e 192*C_e cycles —
    instead of being held hostage by the most-loaded expert.
  - Device (core j), for each expert e with C_e tokens, all transposed
    layout so matmuls use natural weight layouts:
        gateT[i, c] = sum_h Wg[h, j*512+i] * xT[h, c]   (i in [0,512))
        upT   likewise
        hT    = gelu_tanh(gateT) * upT        [512, C]  fp16 in SBUF
        yT[h, c] = sum_i Wd[j*512+i, h] * hT[i, c]      [2048, C] partial!
    Emission is software-pipelined: the down-projection of expert e-1 is
    woven between the gate/up tiles of expert e so the PE never waits for
    the gelu/mul (DVE/ACT) results of the expert it just finished.
  - Host: combine — sum the 8 per-core partial yT (fp32), then
    out[t] += route[t,e] * yT_e[:, pos].T with route matching the
    reference's scatter-add exactly.
"""

import numpy as np

import concourse.bass as bass
import concourse.mybir as mybir
import concourse.tile as tile
from concourse import bacc


def _install_ntff_hook_shim():
    """The agent image's `antenv` lacks `axon_hooks`, which bass_utils
    imports unconditionally when tracing under axon.  Provide the module
    and register the ctypes-based NTFF profile hook so BASS_TRACE=1 yields
    real HW profiles.  Degrades silently if anything is missing."""
    import sys
    import types

    try:
        import antenv

        try:
            from antenv import axon_hooks  # noqa: F401

            return
        except ImportError:
            pass
        mod = types.ModuleType("antenv.axon_hooks")
        mod._hook = None
        mod.set_axon_ntff_profile_hook = lambda h: setattr(mod, "_hook", h)
        mod.get_axon_ntff_profile_hook = lambda: mod._hook
        sys.modules["antenv.axon_hooks"] = mod
        antenv.axon_hooks = mod
        import os

        so_path = "/opt/axon/libaxon_pjrt.so"
        if os.path.exists(so_path):
            from trn_agent_boot.trn_boot import _ntff_profile_via_ctypes

            mod._hook = _ntff_profile_via_ctypes(so_path)
    except Exception:
        pass


_install_ntff_hook_shim()

from concourse.bass_utils import run_bass_kernel_spmd

H = 2048
I = 4096
E = 8
P = 128
NCORES = 8
ISL = I // NCORES  # 512-wide intermediate slice per core
MI = ISL // P  # 4 gate/up output tiles per expert
KH = H // P  # 16 contraction chunks for gate/up
KI = ISL // P  # 4 contraction chunks for down
MH = H // P  # 16 down output tiles
CMAX = 504  # max token-columns per work item (PSUM bank = 512 fp32)
CPAD = 2
WBLK = MI * KH * P  # 8192 weight cols per expert (gate/up); same for down
F32 = mybir.dt.float32
F16 = mybir.dt.float16

# Results of the last device run (for test harnesses to inspect profiling).
LAST_RESULTS = None

_PROGRAM_CACHE: dict[tuple, "bass.Bass"] = {}


def _build_program(items: tuple[tuple[int, int], ...]) -> "bass.Bass":
    """Bass program for one core: for each (expert, C) item, the full expert
    MLP on its I-slice.  Identical across cores (weights differ)."""
    n = len(items)
    xcols = sum(KH * c for _, c in items)
    ycols = sum(MH * c for _, c in items)

    nc = bacc.Bacc("TRN2", target_bir_lowering=False)

    XT = nc.dram_tensor("XT", [P, xcols], F16, kind="ExternalInput")
    WG = nc.dram_tensor("WG", [P, E * WBLK], F16, kind="ExternalInput")
    WU = nc.dram_tensor("WU", [P, E * WBLK], F16, kind="ExternalInput")
    WD = nc.dram_tensor("WD", [P, E * WBLK], F16, kind="ExternalInput")
    Y = nc.dram_tensor("Y", [P, ycols], F16, kind="ExternalOutput")

    XT_a, WG_a, WU_a, WD_a, Y_a = XT.ap(), WG.ap(), WU.ap(), WD.ap(), Y.ap()

    gelu = mybir.ActivationFunctionType.Gelu_apprx_tanh

    xoffs, yoffs = [], []
    xo = yo = 0
    for _, c in items:
        xoffs.append(xo)
        yoffs.append(yo)
        xo += KH * c
        yo += MH * c

    with tile.TileContext(nc) as tc:
        with (
            tc.tile_pool(name="xpool", bufs=2) as xpool,
            tc.tile_pool(name="hpool", bufs=2) as hpool,
            tc.tile_pool(name="wgpool", bufs=2) as wgpool,
            tc.tile_pool(name="wupool", bufs=2) as wupool,
            tc.tile_pool(name="wdpool", bufs=2) as wdpool,
            tc.tile_pool(name="ypool", bufs=2) as ypool,
            tc.tile_pool(name="gpool", bufs=3) as gpool,
            tc.tile_pool(name="warm", bufs=1) as warm_pool,
            tc.tile_pool(name="psumgu", bufs=2, space="PSUM") as psum_gu,
            tc.tile_pool(name="psumd", bufs=2, space="PSUM") as psum_d_pool,
            tc.tile_pool(name="psumw", bufs=1, space="PSUM") as psum_warm,
        ):
            # --- PE warm-up: dummy matmuls on zeros trip the HAM clock-gate
            # to 8/8 while the first DMAs land (~3.4us budget).
            wz = warm_pool.tile([P, P], F16)
            xz = warm_pool.tile([P, P], F16)
            nc.vector.memset(wz, 0.0)
            nc.vector.memset(xz, 0.0)
            psum_w = psum_warm.tile([P, P], F32, tag="warm")
            for _ in range(48):
                nc.tensor.matmul(psum_w, wz, xz, start=True, stop=True)

            tiles = [None] * n  # per item: (xsb, wg_v, wu_v, wd_v)
            ysbs = [None] * n

            def emit_loads(i, first):
                e, C = items[i]
                xsb = xpool.tile([P, KH, C], F16, tag="x", name=f"x_{i}")
                xflat = xsb.rearrange("p k c -> p (k c)")
                xa = XT_a[:, xoffs[i] : xoffs[i] + KH * C]
                wg_t = wgpool.tile([P, WBLK], F16, tag="wg", name=f"wg_{i}")
                wu_t = wupool.tile([P, WBLK], F16, tag="wu", name=f"wu_{i}")
                wd_t = wdpool.tile([P, WBLK], F16, tag="wd", name=f"wd_{i}")
                ga = WG_a[:, e * WBLK : (e + 1) * WBLK]
                ua = WU_a[:, e * WBLK : (e + 1) * WBLK]
                da = WD_a[:, e * WBLK : (e + 1) * WBLK]

                def dx(a, b):  # x chunks [a,b) of KH
                    nc.sync.dma_start(
                        out=xflat[:, a * C : b * C], in_=xa[:, a * C : b * C]
                    )

                def dw(wt, sa, a, b):  # weight cols [a,b) of WBLK
                    nc.sync.dma_start(out=wt[:, a:b], in_=sa[:, a:b])

                KP = KH * P  # 2048 cols per gate/up m-tile
                if first:
                    # need-ordered fine chunks: the first gate matmuls can
                    # start after just x[k0] + wg[m0, k0:8] have landed
                    dx(0, 1)
                    dw(wg_t, ga, 0, KP // 2)
                    dx(1, 2)
                    dw(wg_t, ga, KP // 2, KP)
                    dx(2, 4)
                    dx(4, 8)
                    dw(wu_t, ua, 0, KP)
                    dx(8, 16)
                    for m in range(1, MI):
                        dw(wg_t, ga, m * KP, (m + 1) * KP)
                        dw(wu_t, ua, m * KP, (m + 1) * KP)
                    dw(wd_t, da, 0, WBLK // 2)
                    dw(wd_t, da, WBLK // 2, WBLK)
                else:
                    for a, b in zip([0, 4, 10], [4, 10, 16]):
                        dx(a, b)
                    for a, b in zip([0, WBLK // 2], [WBLK // 2, WBLK]):
                        dw(wg_t, ga, a, b)
                        dw(wu_t, ua, a, b)
                    dw(wd_t, da, 0, WBLK)
                tiles[i] = (
                    xsb,
                    wg_t.rearrange("p (m k i) -> p m k i", m=MI, k=KH),
                    wu_t.rearrange("p (m k i) -> p m k i", m=MI, k=KH),
                    wd_t.rearrange("p (m k i) -> p m k i", m=MH, k=KI),
                )

            def emit_down_chunk(i, m2_range):
                """Down-projection tiles m2_range of item i + drains."""
                e, C = items[i]
                wd_v = tiles[i][3]
                hsb = hsbs[i]
                if ysbs[i] is None:
                    ysbs[i] = ypool.tile([P, MH, C], F16, tag="y", name=f"y_{i}")
                ysb = ysbs[i]
                for m2 in m2_range:
                    psum_d = psum_d_pool.tile([P, C], F32, tag="d")
                    for k2 in range(KI):
                        nc.tensor.matmul(
                            psum_d,
                            wd_v[:, m2, k2, :],
                            hsb[:, k2, :],
                            start=(k2 == 0),
                            stop=(k2 == KI - 1),
                        )
                    nc.vector.tensor_copy(ysb[:, m2, :], psum_d)

            def emit_y_dma(i, lo, hi):
                e, C = items[i]
                yflat = ysbs[i].rearrange("p m c -> p (m c)")
                # scalar-engine HWDGE queue: keeps writebacks off the sync
                # engine, whose queue carries the latency-critical loads
                nc.scalar.dma_start(
                    out=Y_a[:, yoffs[i] + lo * C : yoffs[i] + hi * C],
                    in_=yflat[:, lo * C : hi * C],
                )

            hsbs = [None] * n
            down_sched = [range(0, 4), range(4, 8), range(8, 12), range(12, 16)]

            emit_loads(0, True)

            for i in range(n):
                e, C = items[i]
                if i + 1 < n:
                    emit_loads(i + 1, False)
                xsb, wg_v, wu_v, _ = tiles[i]
                hsb = hpool.tile([P, KI, C], F16, tag="h", name=f"h_{i}")
                hsbs[i] = hsb
                for m in range(MI):
                    psum_g = psum_gu.tile([P, C], F32, tag="g")
                    psum_u = psum_gu.tile([P, C], F32, tag="u")
                    for k in range(KH):
                        nc.tensor.matmul(
                            psum_g,
                            wg_v[:, m, k, :],
                            xsb[:, k, :],
                            start=(k == 0),
                            stop=(k == KH - 1),
                        )
                    for k in range(KH):
                        nc.tensor.matmul(
                            psum_u,
                            wu_v[:, m, k, :],
                            xsb[:, k, :],
                            start=(k == 0),
                            stop=(k == KH - 1),
                        )
                    tg = gpool.tile([P, C], F32, tag="gelu", name=f"g_{i}_{m}")
                    nc.scalar.activation(tg, psum_g, gelu)
                    nc.vector.tensor_mul(hsb[:, m, :], tg, psum_u)
                    if i > 0:
                        emit_down_chunk(i - 1, down_sched[m])
                        if m == 1:
                            emit_y_dma(i - 1, 0, 8)
                        elif m == 3:
                            emit_y_dma(i - 1, 8, 16)
                # release previous item's tiles happens via pool cycling

            # tail: down-projection of the last item
            for q in range(4):
                emit_down_chunk(n - 1, down_sched[q])
                emit_y_dma(n - 1, 4 * q, 4 * q + 4)

    nc.compile()
    return nc


def _get_program(items) -> "bass.Bass":
    key = tuple(items)
    if key not in _PROGRAM_CACHE:
        _PROGRAM_CACHE[key] = _build_program(key)
    return _PROGRAM_CACHE[key]


def _pack_wgu(w16):
    """[E, H, I] fp16 -> [NCORES, P, E*WBLK]: per core j, expert-block layout
    cols = ((e*MI + m)*KH + k)*P + i  with value Wg[e][k*P+p, j*ISL+m*P+i]."""
    a = w16.reshape(E, KH, P, NCORES, MI, P).transpose(3, 2, 0, 4, 1, 5)
    return np.ascontiguousarray(a).reshape(NCORES, P, E * WBLK)


def _pack_wd(w16):
    """[E, I, H] fp16 -> [NCORES, P, E*WBLK]: cols ((e*MH+m2)*KI+k2)*P + hh
    with value Wd[e][j*ISL + k2*P + p, m2*P + hh]."""
    a = w16.reshape(E, NCORES, KI, P, MH, P).transpose(1, 3, 0, 4, 2, 5)
    return np.ascontiguousarray(a).reshape(NCORES, P, E * WBLK)


def kernel(x, selected_experts, routing_weights, Wg, Wu, Wd):
    global LAST_RESULTS
    x = np.asarray(x, dtype=np.float32)
    se = np.asarray(selected_experts).astype(np.int64)
    rw = np.asarray(routing_weights).astype(np.float32)
    Wg = np.asarray(Wg, dtype=np.float32)
    Wu = np.asarray(Wu, dtype=np.float32)
    Wd = np.asarray(Wd, dtype=np.float32)

    T, K = se.shape
    assert x.shape == (T, H) and Wg.shape == (E, H, I) and Wd.shape == (E, I, H)

    # Dense route matrix, identical to the reference's scatter-add (merges
    # duplicate expert picks within a token by summing their weights).
    flat_t = np.repeat(np.arange(T), K)
    flat_e = se.ravel()
    route = np.zeros((T, E), np.float32)
    np.add.at(route, (flat_t, flat_e), rw.ravel())
    present = np.zeros((T, E), bool)
    present[flat_t, flat_e] = True

    # Work items: (expert, token-index chunk), chunks capped at CMAX columns,
    # padded to a multiple of CPAD.  Largest first (shortest tail last).
    work = []
    for e in range(E):
        ix = np.nonzero(present[:, e])[0]
        for s in range(0, len(ix), CMAX):
            chunk = ix[s : s + CMAX]
            cpad = max(CPAD, -(-len(chunk) // CPAD) * CPAD)
            work.append((e, chunk, cpad))
    work.sort(key=lambda w: -w[2])

    items = tuple((e, c) for e, _, c in work)
    nc = _get_program(items)

    xoffs, yoffs = [], []
    xo = yo = 0
    for _, c in items:
        xoffs.append(xo)
        yoffs.append(yo)
        xo += KH * c
        yo += MH * c

    # --- pack inputs
    XT = np.zeros((P, xo), np.float16)
    for (e, ix, c), xof in zip(work, xoffs):
        blk = np.zeros((P, KH, c), np.float16)
        if len(ix):
            # x[ix].T: [H, Ca] -> [KH, P, Ca] -> [P, KH, Ca]
            blk[:, :, : len(ix)] = (
                x[ix].T.astype(np.float16).reshape(KH, P, len(ix)).transpose(1, 0, 2)
            )
        XT[:, xof : xof + KH * c] = blk.reshape(P, KH * c)

    WGp = _pack_wgu(Wg.astype(np.float16))
    WUp = _pack_wgu(Wu.astype(np.float16))
    WDp = _pack_wd(Wd.astype(np.float16))

    in_maps = [
        {"XT": XT, "WG": WGp[j], "WU": WUp[j], "WD": WDp[j]} for j in range(NCORES)
    ]
    res = run_bass_kernel_spmd(nc, in_maps, core_ids=list(range(NCORES)))
    LAST_RESULTS = res

    # --- combine: sum partial yT across cores, then weighted scatter-add
    Ysum = np.zeros((P, yo), np.float32)
    for j in range(NCORES):
        Ysum += res.results[j]["Y"].astype(np.float32)

    out = np.zeros((T, H), np.float32)
    for (e, ix, c), yof in zip(work, yoffs):
        if len(ix) == 0:
            continue
        blk = Ysum[:, yof : yof + MH * c].reshape(P, MH, c)[:, :, : len(ix)]
        # [p, m2, c] -> [c, m2, p] -> [c, H]
        y_e = np.ascontiguousarray(blk.transpose(2, 1, 0)).reshape(len(ix), H)
        out[ix] += route[ix, e][:, None] * y_e
    return out
